# revision 37
# baseline (speedup 1.0000x reference)
"""Multi-head attention (B=4, S=2048, D=1024, H=16, E=64) on 8 TRN2 NeuronCores.

Sharding: core c = (batch b=c//2, head-group hg=c%2 of 8 heads). Each core
computes Q/K/V for its 8 heads over the full 2048-token sequence (no duplicated
projection work), full attention for those heads, and a PARTIAL output
projection (contraction over its 512 head-dims of wo). The host sums the two
partials per batch and adds the constant (bo + bv @ wo.T) — softmax weights sum
to 1, so the V bias contributes a constant vector through the out projection.

Per-core program (SPMD, identical on all cores), 4 passes of 2 heads:
  - All inputs host-relaid into partition-major SBUF format so DMAs move
    4-16KB contiguous runs per partition; weights are emitted into the DMA
    queue before the bulk of x so pass-0 K/Q can start immediately.
  - V projection for all 8 heads upfront, kept in SBUF as bf16 [tok%128, head,
    tok//128, 128]: col 64 = ones (softmax sums ride in att psum row 64),
    cols 65-127 zero-padding so the att@V weight load is full-width (FWL).
  - pass p: KT/QT [128 he(2 heads), 2048 tok] = w.T @ xT + bias (fp32r).
    scores: per (tq-tile of 512, tk-tile of 128): TWO row-packed K=64 matmuls
    (head A rows 0-63, head B rows 64-127 via auto tile_position) run
    CONCURRENTLY on disjoint PE row-groups, writing separate PSUM banks.
    exp on ScalarE from PSUM [128, 2, 512] with scale=1/8 (max-subtraction
    skipped: |score/8| <= ~12, fp32-safe), output bf16 SBUF.
    att[128, 512] += V_pad_h.T @ exp accumulated over tk (row 64 = sums).
  - The scores+exp stage is SOFTWARE-PIPELINED 2 iterations ahead of the
    att@V stage so the exp stream crosses tqt/pass boundaries without psum
    refill bubbles. ScalarE (exp) is the throughput-limiting engine.
  - normalize, deferred one tq-tile and kept entirely off the PE stream:
    sums copy + reciprocal_approx_fast on DVE (SBUF source only — the custom
    op misreads PSUM), partition-broadcast on the otherwise-idle GpSimd,
    multiply into attT bf16 on DVE.
  - out partial [2048, 1024] = attT(bf16) @ wo_slice(bf16), no bias (host).

K/Q projections for pass p+1 and the final out-projection are emitted as
generators yielding every 2 matmuls, drained one micro-step per t-iteration
(the PE sequencer is in-order — fine-grained program-order interleaving is
what hides them under the exp stream without starving it).
"""

import numpy as np

import concourse.bass as bass
import concourse.mybir as mybir
import concourse.tile as tile
from concourse import bacc
from concourse.bass_utils import run_bass_kernel_spmd

FP32 = mybir.dt.float32
FP32R = mybir.dt.float32r
BF16 = mybir.dt.bfloat16
AF = mybir.ActivationFunctionType

B, S, D, H, E = 4, 2048, 1024, 16, 64
NCORES = 8
HPC = 8          # heads per core
NP = 4           # passes (2 heads each)
NT = S // 128    # tk tiles
NQT = S // 512   # tq tiles
SCALE = 1.0 / float(np.sqrt(E))

_CACHE = {}


def build_nc():
    nc = bacc.Bacc("TRN2", target_bir_lowering=False)

    # All inputs are pre-laid-out on the host in partition-major SBUF format so
    # every DMA has long (4-16KB) contiguous runs per partition.
    xq = nc.dram_tensor("xq", [4, 128, 8, 512], FP32R, kind="ExternalInput")
    wq4 = nc.dram_tensor("wq4", [NP, 128, 8, 128], FP32R, kind="ExternalInput")
    wk4 = nc.dram_tensor("wk4", [NP, 128, 8, 128], FP32R, kind="ExternalInput")
    wvr = nc.dram_tensor("wvr", [128, 8, 512], FP32R, kind="ExternalInput")
    wor = nc.dram_tensor("wor", [128, 4, D], BF16, kind="ExternalInput")
    bqr = nc.dram_tensor("bqr", [1, NP, 128], FP32R, kind="ExternalInput")
    bkr = nc.dram_tensor("bkr", [1, NP, 128], FP32R, kind="ExternalInput")
    out = nc.dram_tensor("out", [S, D], FP32, kind="ExternalOutput")

    with tile.TileContext(nc) as tc:
        with (
            tc.tile_pool(name="xt", bufs=1) as xt_pool,
            tc.tile_pool(name="bigw", bufs=1) as bigw_pool,
            tc.tile_pool(name="kqw", bufs=2) as kqw_pool,
            tc.tile_pool(name="ktqt", bufs=2) as ktqt_pool,
            tc.tile_pool(name="vp", bufs=1) as v_pool,
            tc.tile_pool(name="expp", bufs=4) as exp_pool,
            tc.tile_pool(name="attT", bufs=1) as attT_pool,
            tc.tile_pool(name="small", bufs=2) as small_pool,
            tc.tile_pool(name="misc", bufs=1) as misc_pool,
            tc.tile_pool(name="ps", bufs=2, space="PSUM") as ps_pool,
            tc.tile_pool(name="ps_att", bufs=2, space="PSUM") as att_pool,
        ):
            # ---- persistent tiles + initial DMAs ----
            # xT is chunked (s-quarter major, then d-tile) so the pass-0 K/Q
            # projection and V projection can start as soon as their token
            # range has landed instead of waiting for the full 8MB.
            # DMA emission order = queue FIFO order: small weight/bias
            # tensors first (pass-0 K/Q needs them immediately), then x
            # quarter 0, V weights, and the remaining x quarters.
            xt_sb = xt_pool.tile([128, 4, 8, 512], FP32R, tag="xt")
            wv_sb = bigw_pool.tile([128, 8, 512], FP32R, tag="bigw", name="wv")
            bq_sb = misc_pool.tile([1, NP, 128], FP32R, tag="bq")
            bk_sb = misc_pool.tile([1, NP, 128], FP32R, tag="bk")
            nc.sync.dma_start(out=bq_sb, in_=bqr[:, :, :])
            nc.sync.dma_start(out=bk_sb, in_=bkr[:, :, :])
            ones_fs = misc_pool.tile([1, 512], FP32, tag="onesf")
            nc.vector.memset(ones_fs, 1.0)
            ones_row = misc_pool.tile([1, 512], FP32R, tag="ones")
            nc.vector.tensor_copy(out=ones_row, in_=ones_fs)
            wk0, wq0 = None, None  # set below via kq_dma(0)

            def dma_x_rest():
                nc.sync.dma_start(out=xt_sb[:, 0, :, :], in_=xq[0, :, :, :])
                nc.sync.dma_start(out=wv_sb, in_=wvr[:, :, :])
                for q in range(1, 4):
                    nc.sync.dma_start(out=xt_sb[:, q, :, :], in_=xq[q, :, :, :])

            # V padded to 128 columns (col 64 = ones for softmax sums, rest
            # zero) in bf16: full-width weight loads enable FWL on the att@V
            # stationary operand.
            v_sb = v_pool.tile([128, HPC, NT, 128], BF16, tag="v")
            nc.vector.memset(v_sb[:, :, :, E:], 0.0)
            nc.vector.memset(v_sb[:, :, :, E : E + 1], 1.0)

            attT_sb = attT_pool.tile([128, NP, S], BF16, tag="attT")

            # ---- helpers ----
            bg = []  # background (interleavable) work units
            deferred_norm = []  # normalization finishers, one tqt behind
            active = [None]  # in-progress background generator

            def drain(n=1):
                # Advance background work by ~n micro-steps (2 matmuls each)
                # so inserted PE bursts stay small.
                for _ in range(n):
                    if active[0] is None:
                        if not bg:
                            return
                        active[0] = bg.pop(0).gen()
                    try:
                        next(active[0])
                    except StopIteration:
                        active[0] = None

            def drain_norm():
                while deferred_norm:
                    deferred_norm.pop(0)()

            def kq_dma(p):
                wk_sb = kqw_pool.tile([128, 8, 128], FP32R, tag="wk", name=f"wk{p}")
                wq_sb = kqw_pool.tile([128, 8, 128], FP32R, tag="wq", name=f"wq{p}")
                nc.sync.dma_start(out=wk_sb, in_=wk4[p, :, :, :])
                nc.sync.dma_start(out=wq_sb, in_=wq4[p, :, :, :])
                return wk_sb, wq_sb

            def make_ktqt(p):
                kt = ktqt_pool.tile([128, S], FP32R, tag="kt", name=f"kt{p}")
                qt = ktqt_pool.tile([128, S], FP32R, tag="qt", name=f"qt{p}")
                return kt, qt

            def proj_unit(w_sb, dst, tb, bias_sb, p, nm):
                def gen():
                    ps = ps_pool.tile([128, 512], FP32, tag="sc", name=f"ps{nm}")
                    for k in range(8):
                        nc.tensor.matmul(
                            out=ps,
                            lhsT=w_sb[:, k, :],
                            rhs=xt_sb[:, tb, k, :],
                            start=(k == 0),
                            stop=False,
                        )
                        if k % 2 == 1 and k < 7:
                            yield
                    # bias as a rank-1 matmul (bias[he] outer ones[tok]), so the
                    # psum->SBUF mover is a plain ScalarE copy: it slots between
                    # exps and releases the score-psum rotation without waiting
                    # in the DVE FIFO.
                    nc.tensor.matmul(
                        out=ps,
                        lhsT=bias_sb[0:1, p, :],
                        rhs=ones_row,
                        start=False,
                        stop=True,
                    )
                    nc.scalar.copy(
                        out=dst[:, tb * 512 : (tb + 1) * 512],
                        in_=ps,
                    )

                def run():
                    for _ in gen():
                        pass
                run.gen = gen
                return run

            def v_unit(t):
                def run():
                    ps = ps_pool.tile([128, 512], FP32, tag="sc", name=f"psv{t}")
                    for k in range(8):
                        nc.tensor.matmul(
                            out=ps,
                            lhsT=xt_sb[:, t // 4, k, (t % 4) * 128 : (t % 4 + 1) * 128],
                            rhs=wv_sb[:, k, :],
                            start=(k == 0),
                            stop=(k == 7),
                        )
                    # ScalarE copy: keeps the congested prologue off the DVE,
                    # whose FIFO gates the score-psum rotation via bias moves.
                    nc.scalar.copy(
                        out=v_sb[:, :, t, :E],
                        in_=ps.rearrange("p (h e) -> p h e", e=E),
                    )
                return run

            wo_sb = None

            def out_unit(tokt, nd):
                def gen():
                    ps = ps_pool.tile([128, 512], FP32, tag="sc", name=f"pso{tokt}_{nd}")
                    for blk in range(4):
                        nc.tensor.matmul(
                            out=ps,
                            lhsT=attT_sb[:, blk, tokt * 128 : (tokt + 1) * 128],
                            rhs=wo_sb[:, blk, nd * 512 : (nd + 1) * 512],
                            start=(blk == 0),
                            stop=(blk == 3),
                        )
                        if blk == 1:
                            yield
                    osb = small_pool.tile(
                        [128, 512], FP32, tag="ostg", bufs=2, name=f"osb{tokt}_{nd}"
                    )
                    nc.vector.tensor_copy(out=osb, in_=ps)
                    nc.sync.dma_start(
                        out=out[tokt * 128 : (tokt + 1) * 128, nd * 512 : (nd + 1) * 512],
                        in_=osb,
                    )

                def run():
                    for _ in gen():
                        pass
                run.gen = gen
                return run

            # ---- pass 0 K/Q projection: lazy, paced to xT chunk arrival ----
            wk0, wq0 = kq_dma(0)
            dma_x_rest()
            kt, qt = make_ktqt(0)
            kt_units = [proj_unit(wk0, kt, tb, bk_sb, 0, f"k0{tb}") for tb in range(4)]
            qt_units = [proj_unit(wq0, qt, tb, bq_sb, 0, f"q0{tb}") for tb in range(4)]
            kt_units[0]()
            qt_units[0]()

            vunits = [v_unit(t) for t in range(NT)]
            vunits[0]()
            vunits[1]()

            # ---- passes: software-pipelined ----
            # The scores+exp stage LEADS the att@V stage by LEAD iterations so
            # the exp stream flows across tqt/pass boundaries without the
            # refill bubble (first scores of a tqt alias the psum slot of the
            # second-to-last iteration, already freed).
            LEAD = 2
            iters = [(p, tqt, t) for p in range(NP) for tqt in range(NQT) for t in range(NT)]
            kts = {0: kt}
            qts = {0: qt}
            exp_by_i = {}
            att_cur = {}

            def make_norm_tail(attA, attB, p, tqt):
                def norm_tail():
                    for hh, att_ps in ((0, attA), (1, attB)):
                        sums_sb = small_pool.tile(
                            [1, 512], FP32, tag="sums", bufs=1, name=f"sum{p}{tqt}{hh}"
                        )
                        nc.vector.tensor_copy(out=sums_sb, in_=att_ps[E : E + 1, :])
                        recr = small_pool.tile(
                            [1, 512], FP32, tag="recr", bufs=1, name=f"recr{p}{tqt}{hh}"
                        )
                        nc.vector.reciprocal_approx_fast(out=recr, in_=sums_sb)
                        rb_sb = small_pool.tile(
                            [64, 512], FP32, tag="rb", bufs=2, name=f"rbs{p}{tqt}{hh}"
                        )
                        nc.gpsimd.partition_broadcast(rb_sb, recr)
                        nc.vector.tensor_mul(
                            out=attT_sb[
                                hh * 64 : (hh + 1) * 64, p, tqt * 512 : (tqt + 1) * 512
                            ],
                            in0=att_ps[0:E, :],
                            in1=rb_sb,
                        )
                    if p == NP - 1:
                        for tokt in range(tqt * 4, (tqt + 1) * 4):
                            for nd in range(2):
                                bg.append(out_unit(tokt, nd))
                return norm_tail

            wo_sb = None
            for i in range(len(iters) + LEAD):
                # ---- leading stage: scores + exp ----
                if i < len(iters):
                    p, tqt, t = iters[i]
                    if tqt == 0 and t == 0:
                        # Pass start (leading): ensure this pass's KT/QT writes
                        # are all emitted before its scores read them.
                        drain(4 * len(bg) + 4)
                        if p < NP - 1:
                            wkp, wqp = kq_dma(p + 1)
                            ktn, qtn = make_ktqt(p + 1)
                            kts[p + 1], qts[p + 1] = ktn, qtn
                            for tb in range(4):
                                bg.append(
                                    proj_unit(wkp, ktn, tb, bk_sb, p + 1, f"k{p+1}{tb}")
                                )
                            for tb in range(4):
                                bg.append(
                                    proj_unit(wqp, qtn, tb, bq_sb, p + 1, f"q{p+1}{tb}")
                                )
                        else:
                            wo_sb = bigw_pool.tile([128, 4, D], BF16, tag="bigw", name="wo")
                            nc.sync.dma_start(out=wo_sb, in_=wor[:, :, :])
                    if p == 0 and tqt == 0 and t in (3, 7, 11):
                        kt_units[t // 4 + 1]()
                    if p == 0 and t == 0 and tqt >= 1:
                        qt_units[tqt]()
                    kt, qt = kts[p], qts[p]
                    ps_s = ps_pool.tile(
                        [128, 2, 512], FP32, tag="sc", name=f"pss{p}{tqt}{t}"
                    )
                    nc.tensor.matmul(
                        out=ps_s[:, 0, :],
                        lhsT=kt[0:64, t * 128 : (t + 1) * 128],
                        rhs=qt[0:64, tqt * 512 : (tqt + 1) * 512],
                        start=True,
                        stop=True,
                    )
                    nc.tensor.matmul(
                        out=ps_s[:, 1, :],
                        lhsT=kt[64:128, t * 128 : (t + 1) * 128],
                        rhs=qt[64:128, tqt * 512 : (tqt + 1) * 512],
                        start=True,
                        stop=True,
                    )
                    exp_t = exp_pool.tile(
                        [128, 2, 512], BF16, tag="exp", name=f"exp{p}{tqt}{t}"
                    )
                    nc.scalar.activation(out=exp_t, in_=ps_s, func=AF.Exp, scale=SCALE)
                    exp_by_i[i] = exp_t

                # ---- trailing stage: att@V + background/norm pacing ----
                j = i - LEAD
                if j < 0:
                    continue
                p, tqt, t = iters[j]
                if t == 6:
                    drain_norm()
                if p == 0 and tqt == 0:
                    if t + 2 < NT:
                        vunits[t + 2]()
                elif p == NP - 1:
                    drain(2)
                else:
                    drain(1)
                if t == 0:
                    att_cur[0] = att_pool.tile(
                        [128, 512], FP32, tag="attA", name=f"attA{p}{tqt}"
                    )
                    att_cur[1] = att_pool.tile(
                        [128, 512], FP32, tag="attB", name=f"attB{p}{tqt}"
                    )
                exp_t = exp_by_i.pop(j)
                nc.tensor.matmul(
                    out=att_cur[0],
                    lhsT=v_sb[:, 2 * p, t, :],
                    rhs=exp_t[:, 0, :],
                    start=(t == 0),
                    stop=(t == NT - 1),
                )
                nc.tensor.matmul(
                    out=att_cur[1],
                    lhsT=v_sb[:, 2 * p + 1, t, :],
                    rhs=exp_t[:, 1, :],
                    start=(t == 0),
                    stop=(t == NT - 1),
                )
                if t == NT - 1:
                    deferred_norm.append(make_norm_tail(att_cur[0], att_cur[1], p, tqt))

            drain_norm()
            drain(4 * len(bg) + 8)

    nc.compile()
    return nc


def kernel(x, wq, bq, wk, bk, wv, bv, wo, bo, trace=False):
    import ml_dtypes

    x = np.asarray(x, dtype=np.float32)
    wq = np.asarray(wq, dtype=np.float32)
    bq = np.asarray(bq, dtype=np.float32)
    wk = np.asarray(wk, dtype=np.float32)
    bk = np.asarray(bk, dtype=np.float32)
    wv = np.asarray(wv, dtype=np.float32)
    bv = np.asarray(bv, dtype=np.float32)
    wo = np.asarray(wo, dtype=np.float32)
    bo = np.asarray(bo, dtype=np.float32)

    if "nc" not in _CACHE:
        _CACHE["nc"] = build_nc()
    nc = _CACHE["nc"]

    wo_T = np.ascontiguousarray(wo.T)  # [in 1024, out 1024]
    # softmax weights sum to 1 => V-bias contributes (bv @ wo.T) per row; fold
    # with bo and add on host.
    const_vec = bo + bv.reshape(-1) @ wo_T

    # Host-side relayout into partition-major SBUF format (long DMA runs).
    def w_relayout(w_c):  # [1024, 512] -> [4 pass, 128 p, 8 t, 128 he]
        return np.ascontiguousarray(
            w_c.reshape(8, 128, 4, 128).transpose(2, 1, 0, 3)
        )

    hg_maps = []
    for hg in range(2):
        hs = slice(hg * 8, (hg + 1) * 8)
        wq_c = wq[hs].transpose(1, 0, 2).reshape(D, 512)
        wk_c = wk[hs].transpose(1, 0, 2).reshape(D, 512)
        wv_c = wv[hs].transpose(1, 0, 2).reshape(D, 512)
        wo_c = wo_T[hg * 512 : (hg + 1) * 512, :]
        hg_maps.append({
            "wq4": w_relayout(wq_c),
            "wk4": w_relayout(wk_c),
            "wvr": np.ascontiguousarray(wv_c.reshape(8, 128, 512).transpose(1, 0, 2)),
            "wor": np.ascontiguousarray(
                wo_c.reshape(4, 128, D).transpose(1, 0, 2)
            ).astype(ml_dtypes.bfloat16),
            "bqr": np.ascontiguousarray(bq[hs].reshape(8, 64).reshape(1, 4, 128)),
            "bkr": np.ascontiguousarray(bk[hs].reshape(8, 64).reshape(1, 4, 128)),
        })
    # xq[q, p, t, s'] = x[b].T[t*128+p, q*512+s']
    xqs = [
        np.ascontiguousarray(x[b].T.reshape(8, 128, 4, 512).transpose(2, 1, 0, 3))
        for b in range(B)
    ]

    in_maps = []
    for c in range(NCORES):
        b, hg = c // 2, c % 2
        m = dict(hg_maps[hg])
        m["xq"] = xqs[b]
        in_maps.append(m)

    res = run_bass_kernel_spmd(nc, in_maps, list(range(NCORES)), trace=trace)

    out = np.empty((B, S, D), dtype=np.float32)
    for b in range(B):
        out[b] = res.results[2 * b]["out"]
        out[b] += res.results[2 * b + 1]["out"]
        out[b] += const_vec[None, :]
    if trace:
        return out, res
    return out


# revision 39
# speedup vs baseline: 1.1942x; 1.1942x over previous
"""Multi-head attention (B=4, S=2048, D=1024, H=16, E=64) on 8 TRN2 NeuronCores.

Sharding: core c = (batch b=c//2, head-group hg=c%2 of 8 heads). Each core
computes Q/K/V for its 8 heads over the full 2048-token sequence (no duplicated
projection work), full attention for those heads, and a PARTIAL output
projection (contraction over its 512 head-dims of wo). The host sums the two
partials per batch and adds the constant (bo + bv @ wo.T) — softmax weights sum
to 1, so the V bias contributes a constant vector through the out projection.

Per-core program (SPMD, identical on all cores), 4 passes of 2 heads:
  - All inputs host-relaid into partition-major SBUF format so DMAs move
    4-16KB contiguous runs per partition; weights are emitted into the DMA
    queue before the bulk of x so pass-0 K/Q can start immediately.
  - V projection for all 8 heads upfront, kept in SBUF as bf16 [tok%128, head,
    tok//128, 128]: col 64 = ones (softmax sums ride in att psum row 64),
    cols 65-127 zero-padding so the att@V weight load is full-width (FWL).
  - pass p: KT/QT [128 he(2 heads), 2048 tok] = w.T @ xT + bias (fp32r).
    scores: per (tq-tile of 512, tk-tile of 128): TWO row-packed K=64 matmuls
    (head A rows 0-63, head B rows 64-127 via auto tile_position) run
    CONCURRENTLY on disjoint PE row-groups, writing separate PSUM banks.
    exp on ScalarE from PSUM [128, 2, 512] with scale=1/8 (max-subtraction
    skipped: |score/8| <= ~12, fp32-safe), output bf16 SBUF.
    att[128, 512] += V_pad_h.T @ exp accumulated over tk (row 64 = sums).
  - The scores+exp stage is SOFTWARE-PIPELINED 2 iterations ahead of the
    att@V stage so the exp stream crosses tqt/pass boundaries without psum
    refill bubbles. ScalarE (exp) is the throughput-limiting engine.
  - normalize, deferred one tq-tile and kept entirely off the PE stream:
    sums copy + reciprocal_approx_fast on DVE (SBUF source only — the custom
    op misreads PSUM), partition-broadcast on the otherwise-idle GpSimd,
    multiply into attT bf16 on DVE.
  - out partial [2048, 1024] = attT(bf16) @ wo_slice(bf16), no bias (host).

K/Q projections for pass p+1 and the final out-projection are emitted as
generators yielding every 2 matmuls, drained one micro-step per t-iteration
(the PE sequencer is in-order — fine-grained program-order interleaving is
what hides them under the exp stream without starving it).
"""

import numpy as np

import concourse.bass as bass
import concourse.mybir as mybir
import concourse.tile as tile
from concourse import bacc
from concourse.bass_utils import run_bass_kernel_spmd

FP32 = mybir.dt.float32
FP32R = mybir.dt.float32r
BF16 = mybir.dt.bfloat16
AF = mybir.ActivationFunctionType

B, S, D, H, E = 4, 2048, 1024, 16, 64
NCORES = 8
HPC = 8          # heads per core
NP = 4           # passes (2 heads each)
NT = S // 128    # tk tiles
NQT = S // 512   # tq tiles
SCALE = 1.0 / float(np.sqrt(E))

_CACHE = {}


def build_nc():
    nc = bacc.Bacc("TRN2", target_bir_lowering=False)

    # All inputs are pre-laid-out on the host in partition-major SBUF format so
    # every DMA has long (4-16KB) contiguous runs per partition.
    xq = nc.dram_tensor("xq", [4, 128, 8, 512], FP32R, kind="ExternalInput")
    wq4 = nc.dram_tensor("wq4", [NP, 128, 8, 128], FP32R, kind="ExternalInput")
    wk4 = nc.dram_tensor("wk4", [NP, 128, 8, 128], FP32R, kind="ExternalInput")
    wvr = nc.dram_tensor("wvr", [128, 8, 512], FP32R, kind="ExternalInput")
    wor = nc.dram_tensor("wor", [128, 4, D], BF16, kind="ExternalInput")
    bqp = nc.dram_tensor("bqp", [128, NP], FP32, kind="ExternalInput")
    bkp = nc.dram_tensor("bkp", [128, NP], FP32, kind="ExternalInput")
    out = nc.dram_tensor("out", [S, D], FP32, kind="ExternalOutput")

    with tile.TileContext(nc) as tc:
        with (
            tc.tile_pool(name="xt", bufs=1) as xt_pool,
            tc.tile_pool(name="bigw", bufs=1) as bigw_pool,
            tc.tile_pool(name="kqw", bufs=2) as kqw_pool,
            tc.tile_pool(name="ktqt", bufs=2) as ktqt_pool,
            tc.tile_pool(name="vp", bufs=1) as v_pool,
            tc.tile_pool(name="expp", bufs=6) as exp_pool,
            tc.tile_pool(name="attT", bufs=1) as attT_pool,
            tc.tile_pool(name="small", bufs=2) as small_pool,
            tc.tile_pool(name="misc", bufs=1) as misc_pool,
            tc.tile_pool(name="ps", bufs=2, space="PSUM") as ps_pool,
            tc.tile_pool(name="ps_att", bufs=2, space="PSUM") as att_pool,
        ):
            # ---- persistent tiles + initial DMAs ----
            # xT is chunked (s-quarter major, then d-tile) so the pass-0 K/Q
            # projection and V projection can start as soon as their token
            # range has landed instead of waiting for the full 8MB.
            # DMA emission order = queue FIFO order: small weight/bias
            # tensors first (pass-0 K/Q needs them immediately), then x
            # quarter 0, V weights, and the remaining x quarters.
            xt_sb = xt_pool.tile([128, 4, 8, 512], FP32R, tag="xt")
            wv_sb = bigw_pool.tile([128, 8, 512], FP32R, tag="bigw", name="wv")
            bq_sb = misc_pool.tile([128, NP], FP32, tag="bq")
            bk_sb = misc_pool.tile([128, NP], FP32, tag="bk")
            nc.sync.dma_start(out=bq_sb, in_=bqp[:, :])
            nc.sync.dma_start(out=bk_sb, in_=bkp[:, :])
            wk0, wq0 = None, None  # set below via kq_dma(0)

            def dma_x_rest():
                for k in range(8):
                    nc.sync.dma_start(out=xt_sb[:, 0, k, :], in_=xq[0, :, k, :])
                nc.sync.dma_start(out=wv_sb, in_=wvr[:, :, :])
                for q in range(1, 4):
                    nc.sync.dma_start(out=xt_sb[:, q, :, :], in_=xq[q, :, :, :])

            # V padded to 128 columns (col 64 = ones for softmax sums, rest
            # zero) in bf16: full-width weight loads enable FWL on the att@V
            # stationary operand.
            v_sb = v_pool.tile([128, HPC, NT, 128], BF16, tag="v")
            nc.vector.memset(v_sb[:, :, :, E:], 0.0)
            nc.vector.memset(v_sb[:, :, :, E : E + 1], 1.0)

            attT_sb = attT_pool.tile([128, NP, S], BF16, tag="attT")

            # ---- helpers ----
            bg = []  # background (interleavable) work units
            deferred_norm = []  # normalization finishers, one tqt behind
            active = [None]  # in-progress background generator

            def drain(n=1):
                # Advance background work by ~n micro-steps (2 matmuls each)
                # so inserted PE bursts stay small.
                for _ in range(n):
                    if active[0] is None:
                        if not bg:
                            return
                        active[0] = bg.pop(0).gen()
                    try:
                        next(active[0])
                    except StopIteration:
                        active[0] = None

            def drain_norm():
                while deferred_norm:
                    deferred_norm.pop(0)()

            def kq_dma(p):
                wk_sb = kqw_pool.tile([128, 8, 128], FP32R, tag="wk", name=f"wk{p}")
                wq_sb = kqw_pool.tile([128, 8, 128], FP32R, tag="wq", name=f"wq{p}")
                nc.sync.dma_start(out=wk_sb, in_=wk4[p, :, :, :])
                nc.sync.dma_start(out=wq_sb, in_=wq4[p, :, :, :])
                return wk_sb, wq_sb

            def make_ktqt(p):
                kt = ktqt_pool.tile([128, S], FP32R, tag="kt", name=f"kt{p}")
                qt = ktqt_pool.tile([128, S], FP32R, tag="qt", name=f"qt{p}")
                return kt, qt

            def proj_unit(w_sb, dst, tb, bias_sb, p, nm):
                def gen():
                    ps = ps_pool.tile([128, 512], FP32, tag="sc", name=f"ps{nm}")
                    for k in range(8):
                        nc.tensor.matmul(
                            out=ps,
                            lhsT=w_sb[:, k, :],
                            rhs=xt_sb[:, tb, k, :],
                            start=(k == 0),
                            stop=(k == 7),
                        )
                        if k % 2 == 1 and k < 7:
                            yield
                    nc.vector.tensor_scalar_add(
                        out=dst[:, tb * 512 : (tb + 1) * 512],
                        in0=ps,
                        scalar1=bias_sb[:, p : p + 1],
                    )

                def run():
                    for _ in gen():
                        pass
                run.gen = gen
                return run

            def v_unit(t):
                def run():
                    ps = ps_pool.tile([128, 512], FP32, tag="sc", name=f"psv{t}")
                    for k in range(8):
                        nc.tensor.matmul(
                            out=ps,
                            lhsT=xt_sb[:, t // 4, k, (t % 4) * 128 : (t % 4 + 1) * 128],
                            rhs=wv_sb[:, k, :],
                            start=(k == 0),
                            stop=(k == 7),
                        )
                    # ScalarE copy: keeps the congested prologue off the DVE,
                    # whose FIFO gates the score-psum rotation via bias moves.
                    nc.scalar.copy(
                        out=v_sb[:, :, t, :E],
                        in_=ps.rearrange("p (h e) -> p h e", e=E),
                    )
                return run

            wo_sb = None

            def out_unit(tokt, nd):
                def gen():
                    ps = ps_pool.tile([128, 512], FP32, tag="sc", name=f"pso{tokt}_{nd}")
                    for blk in range(4):
                        nc.tensor.matmul(
                            out=ps,
                            lhsT=attT_sb[:, blk, tokt * 128 : (tokt + 1) * 128],
                            rhs=wo_sb[:, blk, nd * 512 : (nd + 1) * 512],
                            start=(blk == 0),
                            stop=(blk == 3),
                        )
                        if blk == 1:
                            yield
                    osb = small_pool.tile(
                        [128, 512], FP32, tag="ostg", bufs=2, name=f"osb{tokt}_{nd}"
                    )
                    nc.vector.tensor_copy(out=osb, in_=ps)
                    nc.sync.dma_start(
                        out=out[tokt * 128 : (tokt + 1) * 128, nd * 512 : (nd + 1) * 512],
                        in_=osb,
                    )

                def run():
                    for _ in gen():
                        pass
                run.gen = gen
                return run

            # ---- pass 0 K/Q projection: lazy, paced to xT chunk arrival ----
            wk0, wq0 = kq_dma(0)
            dma_x_rest()
            kt, qt = make_ktqt(0)
            kt_units = [proj_unit(wk0, kt, tb, bk_sb, 0, f"k0{tb}") for tb in range(4)]
            qt_units = [proj_unit(wq0, qt, tb, bq_sb, 0, f"q0{tb}") for tb in range(4)]
            kt_units[0]()
            qt_units[0]()

            vunits = [v_unit(t) for t in range(NT)]
            vunits[0]()
            vunits[1]()

            # ---- passes: software-pipelined ----
            # The scores+exp stage LEADS the att@V stage by LEAD iterations so
            # the exp stream flows across tqt/pass boundaries without the
            # refill bubble (first scores of a tqt alias the psum slot of the
            # second-to-last iteration, already freed).
            LEAD = 2
            iters = [(p, tqt, t) for p in range(NP) for tqt in range(NQT) for t in range(NT)]
            kts = {0: kt}
            qts = {0: qt}
            exp_by_i = {}
            att_cur = {}

            def make_norm_tail(attA, attB, p, tqt):
                def norm_tail():
                    for hh, att_ps in ((0, attA), (1, attB)):
                        sums_sb = small_pool.tile(
                            [1, 512], FP32, tag="sums", bufs=1, name=f"sum{p}{tqt}{hh}"
                        )
                        nc.vector.tensor_copy(out=sums_sb, in_=att_ps[E : E + 1, :])
                        recr = small_pool.tile(
                            [1, 512], FP32, tag="recr", bufs=1, name=f"recr{p}{tqt}{hh}"
                        )
                        nc.vector.reciprocal_approx_fast(out=recr, in_=sums_sb)
                        rb_sb = small_pool.tile(
                            [64, 512], FP32, tag="rb", bufs=2, name=f"rbs{p}{tqt}{hh}"
                        )
                        nc.gpsimd.partition_broadcast(rb_sb, recr)
                        nc.vector.tensor_mul(
                            out=attT_sb[
                                hh * 64 : (hh + 1) * 64, p, tqt * 512 : (tqt + 1) * 512
                            ],
                            in0=att_ps[0:E, :],
                            in1=rb_sb,
                        )
                    if p == NP - 1:
                        for tokt in range(tqt * 4, (tqt + 1) * 4):
                            for nd in range(2):
                                bg.append(out_unit(tokt, nd))
                return norm_tail

            wo_sb = None
            for i in range(len(iters) + LEAD):
                # ---- leading stage: scores + exp ----
                if i < len(iters):
                    p, tqt, t = iters[i]
                    if tqt == 0 and t == 0:
                        # Pass start (leading): ensure this pass's KT/QT writes
                        # are all emitted before its scores read them.
                        drain(4 * len(bg) + 4)
                        if p < NP - 1:
                            wkp, wqp = kq_dma(p + 1)
                            ktn, qtn = make_ktqt(p + 1)
                            kts[p + 1], qts[p + 1] = ktn, qtn
                            for tb in range(4):
                                bg.append(
                                    proj_unit(wkp, ktn, tb, bk_sb, p + 1, f"k{p+1}{tb}")
                                )
                            for tb in range(4):
                                bg.append(
                                    proj_unit(wqp, qtn, tb, bq_sb, p + 1, f"q{p+1}{tb}")
                                )
                        else:
                            wo_sb = bigw_pool.tile([128, 4, D], BF16, tag="bigw", name="wo")
                            nc.sync.dma_start(out=wo_sb, in_=wor[:, :, :])
                    if p == 0 and tqt == 0 and t in (3, 7, 11):
                        kt_units[t // 4 + 1]()
                    if p == 0 and t == 0 and tqt >= 1:
                        qt_units[tqt]()
                    kt, qt = kts[p], qts[p]
                    ps_s = ps_pool.tile(
                        [128, 2, 512], FP32, tag="sc", name=f"pss{p}{tqt}{t}"
                    )
                    nc.tensor.matmul(
                        out=ps_s[:, 0, :],
                        lhsT=kt[0:64, t * 128 : (t + 1) * 128],
                        rhs=qt[0:64, tqt * 512 : (tqt + 1) * 512],
                        start=True,
                        stop=True,
                    )
                    nc.tensor.matmul(
                        out=ps_s[:, 1, :],
                        lhsT=kt[64:128, t * 128 : (t + 1) * 128],
                        rhs=qt[64:128, tqt * 512 : (tqt + 1) * 512],
                        start=True,
                        stop=True,
                    )
                    exp_t = exp_pool.tile(
                        [128, 2, 512], BF16, tag="exp", name=f"exp{p}{tqt}{t}"
                    )
                    nc.scalar.activation(out=exp_t, in_=ps_s, func=AF.Exp, scale=SCALE)
                    exp_by_i[i] = exp_t

                # ---- trailing stage: att@V + background/norm pacing ----
                j = i - LEAD
                if j < 0:
                    continue
                p, tqt, t = iters[j]
                if t == 6:
                    drain_norm()
                if p == 0 and tqt == 0:
                    if t + 2 < NT:
                        vunits[t + 2]()
                elif p == NP - 1:
                    drain(3)
                else:
                    drain(1)
                if t == 0:
                    att_cur[0] = att_pool.tile(
                        [128, 512], FP32, tag="attA", name=f"attA{p}{tqt}"
                    )
                    att_cur[1] = att_pool.tile(
                        [128, 512], FP32, tag="attB", name=f"attB{p}{tqt}"
                    )
                exp_t = exp_by_i.pop(j)
                nc.tensor.matmul(
                    out=att_cur[0],
                    lhsT=v_sb[:, 2 * p, t, :],
                    rhs=exp_t[:, 0, :],
                    start=(t == 0),
                    stop=(t == NT - 1),
                )
                nc.tensor.matmul(
                    out=att_cur[1],
                    lhsT=v_sb[:, 2 * p + 1, t, :],
                    rhs=exp_t[:, 1, :],
                    start=(t == 0),
                    stop=(t == NT - 1),
                )
                if t == NT - 1:
                    deferred_norm.append(make_norm_tail(att_cur[0], att_cur[1], p, tqt))

            drain_norm()
            drain(4 * len(bg) + 8)

    nc.compile()
    return nc


def kernel(x, wq, bq, wk, bk, wv, bv, wo, bo, trace=False):
    import ml_dtypes

    x = np.asarray(x, dtype=np.float32)
    wq = np.asarray(wq, dtype=np.float32)
    bq = np.asarray(bq, dtype=np.float32)
    wk = np.asarray(wk, dtype=np.float32)
    bk = np.asarray(bk, dtype=np.float32)
    wv = np.asarray(wv, dtype=np.float32)
    bv = np.asarray(bv, dtype=np.float32)
    wo = np.asarray(wo, dtype=np.float32)
    bo = np.asarray(bo, dtype=np.float32)

    if "nc" not in _CACHE:
        _CACHE["nc"] = build_nc()
    nc = _CACHE["nc"]

    wo_T = np.ascontiguousarray(wo.T)  # [in 1024, out 1024]
    # softmax weights sum to 1 => V-bias contributes (bv @ wo.T) per row; fold
    # with bo and add on host.
    const_vec = bo + bv.reshape(-1) @ wo_T

    # Host-side relayout into partition-major SBUF format (long DMA runs).
    def w_relayout(w_c):  # [1024, 512] -> [4 pass, 128 p, 8 t, 128 he]
        return np.ascontiguousarray(
            w_c.reshape(8, 128, 4, 128).transpose(2, 1, 0, 3)
        )

    hg_maps = []
    for hg in range(2):
        hs = slice(hg * 8, (hg + 1) * 8)
        wq_c = wq[hs].transpose(1, 0, 2).reshape(D, 512)
        wk_c = wk[hs].transpose(1, 0, 2).reshape(D, 512)
        wv_c = wv[hs].transpose(1, 0, 2).reshape(D, 512)
        wo_c = wo_T[hg * 512 : (hg + 1) * 512, :]
        hg_maps.append({
            "wq4": w_relayout(wq_c),
            "wk4": w_relayout(wk_c),
            "wvr": np.ascontiguousarray(wv_c.reshape(8, 128, 512).transpose(1, 0, 2)),
            "wor": np.ascontiguousarray(
                wo_c.reshape(4, 128, D).transpose(1, 0, 2)
            ).astype(ml_dtypes.bfloat16),
            "bqp": np.ascontiguousarray(bq[hs].reshape(8, 64).reshape(4, 128).T),
            "bkp": np.ascontiguousarray(bk[hs].reshape(8, 64).reshape(4, 128).T),
        })
    # xq[q, p, t, s'] = x[b].T[t*128+p, q*512+s']
    xqs = [
        np.ascontiguousarray(x[b].T.reshape(8, 128, 4, 512).transpose(2, 1, 0, 3))
        for b in range(B)
    ]

    in_maps = []
    for c in range(NCORES):
        b, hg = c // 2, c % 2
        m = dict(hg_maps[hg])
        m["xq"] = xqs[b]
        in_maps.append(m)

    res = run_bass_kernel_spmd(nc, in_maps, list(range(NCORES)), trace=trace)

    out = np.empty((B, S, D), dtype=np.float32)
    for b in range(B):
        out[b] = res.results[2 * b]["out"]
        out[b] += res.results[2 * b + 1]["out"]
        out[b] += const_vec[None, :]
    if trace:
        return out, res
    return out
